# revision 13
# baseline (speedup 1.0000x reference)
# MoE router kernel for Trainium2 (Bass/Tile), data-parallel over tokens on 8 cores.
#
# Reference computation (dense MoE, B=8192 D=1024 H=4096 E=4 top_k=2):
#   logits = x @ gate_w.T + gate_b ; probs = softmax(logits) ; top2 -> idx, w
#   xn = layernorm(x) ; xe = xn*ln_w[e] + ln_b[e]
#   h_e = gelu(xe @ w1[e] + b1[e]) ; y_e = h_e @ w2[e] + b2[e]
#   y = sum_k topw_k * y_{idx_k} ; aux = E * sum_e dispatch_frac_e * mean_prob_e
#
# Per-core layout strategy (B_loc = 1024 tokens/core):
#   - token tiles of T=512 (NS=4 subtiles of 128 tokens)
#   - LN stats in natural [b, d] layout; xn transposed via PE to xn^T [d, b]
#   - gate logits from xn^T with per-token sigma/mu correction (fp32 matmuls)
#   - matmul1: h^T[h,b] = w1^T-chunks @ xe^T  (fp32r, 1 cyc/row)
#   - gelu with b1 as per-partition ACT bias
#   - matmul2: y^T[d,b] = w2^T-chunks @ h^T   (fp32r), PSUM-accumulated over h
#   - combine: y^T *= broadcast(c_e[b]) summed over experts on DVE; +b2 once
#   - y^T PE-transposed back to natural layout for contiguous DMA out
#   - aux partial sums (per-expert counts & prob sums) via ones-matmuls

import numpy as np

import concourse.bacc as bacc
import concourse.bass as bass
import concourse.mybir as mybir
import concourse.tile as tile
from concourse import masks

dt = mybir.dt
AF = mybir.ActivationFunctionType
ALU = mybir.AluOpType
AX = mybir.AxisListType

LN_EPS = 1e-5


def build_moe_kernel(B_loc=1024, D=1024, H=4096, E=4, T=512, act_fn=None,
                     debug=False, enable_asserts=False):
    """Build the per-core Bass module. All 8 cores run this same module SPMD."""
    if act_fn is None:
        act_fn = AF.Gelu
    assert T % 128 == 0 and T <= 512
    assert B_loc % T == 0 and D % 128 == 0 and H % 128 == 0
    NT = B_loc // T          # token tiles
    NS = T // 128            # 128-token subtiles per token tile
    ND = D // 128            # contraction chunks over D
    NH = H // 128            # h chunks
    DG = min(4, ND)          # d-chunks per PSUM-resident y group
    NDG = ND // DG
    W1TH = min(512, H)       # w1 DMA tile free size (multiple of 128)
    NW1 = H // W1TH
    HPW = W1TH // 128        # h-chunks per w1 DMA tile
    f32, f32r, i32, u32 = dt.float32, dt.float32r, dt.int32, dt.uint32

    nc = bacc.Bacc("TRN2", target_bir_lowering=False, debug=debug,
                   enable_asserts=enable_asserts)

    # ---- DRAM I/O ----
    x_d = nc.dram_tensor("x", (B_loc, D), f32, kind="ExternalInput")
    gwt_d = nc.dram_tensor("gwt", (D, E), f32, kind="ExternalInput")    # gate_w.T
    gb_d = nc.dram_tensor("gb", (1, E), f32, kind="ExternalInput")
    lnwt_d = nc.dram_tensor("lnwt", (D, E), f32, kind="ExternalInput")  # ln_w.T
    lnbt_d = nc.dram_tensor("lnbt", (D, E), f32, kind="ExternalInput")
    w1_d = nc.dram_tensor("w1", (E, D, H), f32, kind="ExternalInput")
    b1t_d = nc.dram_tensor("b1t", (H, E), f32, kind="ExternalInput")    # b1.T
    w2_d = nc.dram_tensor("w2", (E, H, D), f32, kind="ExternalInput")
    b2_d = nc.dram_tensor("b2", (E, D), f32, kind="ExternalInput")
    sel_d = nc.dram_tensor("sel", (E, E * 128), f32, kind="ExternalInput")

    y_d = nc.dram_tensor("y", (B_loc, D), f32, kind="ExternalOutput")
    probs_d = nc.dram_tensor("probs", (B_loc, E), f32, kind="ExternalOutput")
    topi_d = nc.dram_tensor("top_idx", (B_loc, 2), i32, kind="ExternalOutput")
    topw_d = nc.dram_tensor("top_w", (B_loc, 2), f32, kind="ExternalOutput")
    aux_d = nc.dram_tensor("aux_part", (E, 2), f32, kind="ExternalOutput")

    with tile.TileContext(nc) as tc:
        import contextlib
        with contextlib.ExitStack() as ctx:
            consts = ctx.enter_context(tc.tile_pool(name="consts", bufs=1))
            pxin = ctx.enter_context(tc.tile_pool(name="pxin", bufs=2))
            pstats = ctx.enter_context(tc.tile_pool(name="pstats", bufs=3))
            pgate = ctx.enter_context(tc.tile_pool(name="pgate", bufs=2))
            pxnt = ctx.enter_context(tc.tile_pool(name="pxnt", bufs=1))
            pxet = ctx.enter_context(tc.tile_pool(name="pxet", bufs=1))
            pht = ctx.enter_context(tc.tile_pool(name="pht", bufs=1))
            pyac = ctx.enter_context(tc.tile_pool(name="pyac", bufs=1))
            pw1 = ctx.enter_context(tc.tile_pool(name="pw1", bufs=ND + 4))
            pw2 = ctx.enter_context(tc.tile_pool(name="pw2", bufs=6))
            pcb = ctx.enter_context(tc.tile_pool(name="pcb", bufs=2))
            ptmp = ctx.enter_context(tc.tile_pool(name="ptmp", bufs=3))
            pyout = ctx.enter_context(tc.tile_pool(name="pyout", bufs=2))
            ph = ctx.enter_context(tc.tile_pool(name="ph", bufs=2, space="PSUM"))
            py = ctx.enter_context(tc.tile_pool(name="py", bufs=1, space="PSUM"))
            pms = ctx.enter_context(tc.tile_pool(name="pms", bufs=2, space="PSUM"))

            # ---- constants / preloaded weights ----
            ident = consts.tile([128, 128], f32, tag="ident")
            masks.make_identity(nc, ident[:])
            ones_row = consts.tile([1, T], f32, tag="ones_row")
            nc.vector.memset(ones_row[:], 1.0)
            ones_col = consts.tile([128, 1], f32, tag="ones_col")
            nc.vector.memset(ones_col[:], 1.0)
            iota_e = consts.tile([128, E], u32, tag="iota_e")
            nc.gpsimd.iota(iota_e[:], pattern=[[1, E]], base=0, channel_multiplier=0)

            gwt_sb = consts.tile([128, ND * E], f32, tag="gwt")
            lnw_sb = consts.tile([128, ND * E], f32, tag="lnw")
            lnb_sb = consts.tile([128, ND * E], f32, tag="lnb")
            for c in range(ND):
                sl = np.s_[c * 128:(c + 1) * 128]
                nc.sync.dma_start(gwt_sb[:, c * E:(c + 1) * E], gwt_d[sl, :])
                nc.sync.dma_start(lnw_sb[:, c * E:(c + 1) * E], lnwt_d[sl, :])
                nc.sync.dma_start(lnb_sb[:, c * E:(c + 1) * E], lnbt_d[sl, :])
            b2_sb = consts.tile([E, D], f32, tag="b2")
            nc.sync.dma_start(b2_sb[:], b2_d[:])
            b1t_sb = consts.tile([128, NH * E], f32, tag="b1t")
            for c in range(NH):
                nc.sync.dma_start(b1t_sb[:, c * E:(c + 1) * E],
                                  b1t_d[c * 128:(c + 1) * 128, :])
            gb_sb = consts.tile([1, E], f32, tag="gb")
            nc.sync.dma_start(gb_sb[:], gb_d[:])

            # S_row[e] = sum_d gate_w[e, d] (for the mu correction of gate logits)
            ps_s = pms.tile([1, E], f32, tag="ps_small")
            for c in range(ND):
                nc.tensor.matmul(ps_s[:], lhsT=ones_col[:],
                                 rhs=gwt_sb[:, c * E:(c + 1) * E],
                                 start=(c == 0), stop=(c == ND - 1))
            s_row = consts.tile([1, E], f32, tag="s_row")
            nc.vector.tensor_copy(s_row[:], ps_s[:])

            aux_acc = consts.tile([E, 2], f32, tag="aux_acc")
            nc.vector.memset(aux_acc[:], 0.0)

            # sel_sb[:, e*128:(e+1)*128] is the [E,128] one-hot selector whose
            # matmul against cT broadcasts expert e's coefficient row to all
            # 128 partitions (PE operands must sit at base partition 0).
            sel_sb = consts.tile([E, E * 128], f32, tag="sel")
            nc.sync.dma_start(sel_sb[:], sel_d[:])

            # =======================  main loop over token tiles  ==================
            for t in range(NT):
                xnT = [pxnt.tile([128, T], f32, tag=f"xnt{dc}", name=f"xnt{dc}") for dc in range(ND)]
                cT = pgate.tile([E, T], f32, tag="ct")

                for s in range(NS):
                    row0 = (t * NS + s) * 128
                    rows = np.s_[row0:row0 + 128]
                    bsl = np.s_[s * 128:(s + 1) * 128]

                    # ---- load + LN stats (natural [b, d] layout) ----
                    x_sb = pxin.tile([128, D], f32, tag="x")
                    nc.sync.dma_start(x_sb[:], x_d[rows, :])
                    mu = pstats.tile([128, 1], f32, tag="mu")
                    nc.vector.reduce_sum(mu[:], x_sb[:], axis=AX.X)
                    nc.vector.tensor_scalar_mul(mu[:], mu[:], 1.0 / D)
                    sq = pxin.tile([128, D], f32, tag="sq")
                    ssq = pstats.tile([128, 1], f32, tag="ssq")
                    nc.scalar.activation(sq[:], x_sb[:], AF.Square, accum_out=ssq[:])
                    var = pstats.tile([128, 1], f32, tag="var")
                    nc.vector.tensor_scalar_mul(var[:], ssq[:], 1.0 / D)
                    mu2 = pstats.tile([128, 1], f32, tag="mu2")
                    nc.vector.tensor_tensor(mu2[:], mu[:], mu[:], ALU.mult)
                    nc.vector.tensor_sub(var[:], var[:], mu2[:])
                    nc.vector.tensor_scalar_add(var[:], var[:], LN_EPS)  # var+eps
                    inv = pstats.tile([128, 1], f32, tag="inv")
                    nc.vector.reciprocal(inv[:], var[:])
                    rstd = pstats.tile([128, 1], f32, tag="rstd")
                    nc.scalar.sqrt(rstd[:], inv[:])
                    sigma = pstats.tile([128, 1], f32, tag="sigma")
                    nc.scalar.sqrt(sigma[:], var[:])
                    nmr = pstats.tile([128, 1], f32, tag="nmr")
                    nc.vector.tensor_tensor(nmr[:], mu[:], rstd[:], ALU.mult)
                    nc.vector.tensor_scalar_mul(nmr[:], nmr[:], -1.0)

                    # xn = (x - mu) * rstd
                    xn_sb = pxin.tile([128, D], f32, tag="xn")
                    nc.scalar.activation(xn_sb[:], x_sb[:], AF.Identity,
                                         bias=nmr[:], scale=rstd[:])

                    # mu as a row (for the K=1 gate-correction matmul)
                    ps_mt = pms.tile([1, 128], f32, tag="ps_small")
                    nc.tensor.transpose(ps_mt[:], mu[:], ident[:])
                    mu_row = pstats.tile([1, 128], f32, tag="mu_row")
                    nc.vector.tensor_copy(mu_row[:], ps_mt[:])

                    # ---- transpose xn -> xn^T chunks ----
                    for dc in range(ND):
                        ps_tr = pms.tile([128, 128], f32, tag="ps_small")
                        nc.tensor.transpose(ps_tr[:], xn_sb[:, dc * 128:(dc + 1) * 128],
                                            ident[:])
                        nc.vector.tensor_copy(xnT[dc][:, bsl], ps_tr[:])

                    # ---- gate: logits = sigma*(xn @ gw^T) + mu x S + gb ----
                    ps_g = pms.tile([128, E], f32, tag="ps_small")
                    for dc in range(ND):
                        nc.tensor.matmul(ps_g[:], lhsT=xnT[dc][:, bsl],
                                         rhs=gwt_sb[:, dc * E:(dc + 1) * E],
                                         start=(dc == 0), stop=(dc == ND - 1))
                    l1 = pgate.tile([128, E], f32, tag="l1")
                    nc.scalar.activation(l1[:], ps_g[:], AF.Identity, scale=sigma[:])
                    ps_g2 = pms.tile([128, E], f32, tag="ps_small")
                    nc.tensor.matmul(ps_g2[:], lhsT=mu_row[:], rhs=s_row[:],
                                     start=True, stop=False)
                    nc.tensor.matmul(ps_g2[:], lhsT=ones_row[:, 0:128], rhs=gb_sb[:],
                                     start=False, stop=True)
                    logits = pgate.tile([128, E], f32, tag="logits")
                    nc.vector.tensor_tensor(logits[:], l1[:], ps_g2[:], ALU.add)

                    # ---- softmax ----
                    nmax = pstats.tile([128, 1], f32, tag="nmax")
                    nc.vector.reduce_max(nmax[:], logits[:], axis=AX.X)
                    nc.vector.tensor_scalar_mul(nmax[:], nmax[:], -1.0)
                    probs_sb = pgate.tile([128, E], f32, tag="probs")
                    sume = pstats.tile([128, 1], f32, tag="sume")
                    nc.scalar.activation(probs_sb[:], logits[:], AF.Exp,
                                         bias=nmax[:], accum_out=sume[:])
                    rec = pstats.tile([128, 1], f32, tag="rec")
                    nc.vector.reciprocal(rec[:], sume[:])
                    nc.vector.tensor_scalar_mul(probs_sb[:], probs_sb[:], rec[:])
                    nc.sync.dma_start(probs_d[rows, :], probs_sb[:])

                    # ---- top-2 of E (pad to 8 for the DVE max unit) ----
                    pad8 = pgate.tile([128, 8], f32, tag="pad8")
                    nc.vector.memset(pad8[:], -1.0)
                    nc.vector.tensor_copy(pad8[:, 0:E], probs_sb[:])
                    vmax8 = pgate.tile([128, 8], f32, tag="vmax8")
                    nc.vector.max(vmax8[:], pad8[:])
                    vidx8 = pgate.tile([128, 8], u32, tag="vidx8")
                    nc.vector.max_index(vidx8[:], vmax8[:], pad8[:])
                    s2 = pstats.tile([128, 1], f32, tag="s2")
                    nc.vector.reduce_sum(s2[:], vmax8[:, 0:2], axis=AX.X)
                    nc.vector.tensor_scalar_add(s2[:], s2[:], 1e-9)
                    rec2 = pstats.tile([128, 1], f32, tag="rec2")
                    nc.vector.reciprocal(rec2[:], s2[:])
                    topw = pgate.tile([128, 2], f32, tag="topw")
                    nc.vector.tensor_scalar_mul(topw[:], vmax8[:, 0:2], rec2[:])
                    topi = pgate.tile([128, 2], i32, tag="topi")
                    nc.vector.tensor_copy(topi[:], vidx8[:, 0:2])
                    nc.sync.dma_start(topw_d[rows, :], topw[:])
                    nc.sync.dma_start(topi_d[rows, :], topi[:])

                    # ---- combine coefficients c[b, e] ----
                    is0 = pgate.tile([128, E], f32, tag="is0")
                    is1 = pgate.tile([128, E], f32, tag="is1")
                    a0, b0 = bass.broadcast_tensor_aps(iota_e[:], vidx8[:, 0:1])
                    nc.vector.tensor_tensor(is0[:], a0, b0, ALU.is_equal)
                    a1, b1ap = bass.broadcast_tensor_aps(iota_e[:], vidx8[:, 1:2])
                    nc.vector.tensor_tensor(is1[:], a1, b1ap, ALU.is_equal)
                    c0 = pgate.tile([128, E], f32, tag="c0")
                    nc.vector.tensor_scalar_mul(c0[:], is0[:], topw[:, 0:1])
                    c1 = pgate.tile([128, E], f32, tag="c1")
                    nc.vector.tensor_scalar_mul(c1[:], is1[:], topw[:, 1:2])
                    c_sb = pgate.tile([128, E], f32, tag="c_sb")
                    nc.vector.tensor_tensor(c_sb[:], c0[:], c1[:], ALU.add)

                    # ---- aux partials: counts and prob sums over this subtile ----
                    cnt01 = pgate.tile([128, E], f32, tag="cnt01")
                    nc.vector.tensor_tensor(cnt01[:], is0[:], is1[:], ALU.add)
                    ps_a = pms.tile([E, 1], f32, tag="ps_small")
                    nc.tensor.matmul(ps_a[:], lhsT=cnt01[:], rhs=ones_col[:],
                                     start=True, stop=True)
                    nc.vector.tensor_tensor(aux_acc[:, 0:1], aux_acc[:, 0:1],
                                            ps_a[:], ALU.add)
                    ps_a2 = pms.tile([E, 1], f32, tag="ps_small")
                    nc.tensor.matmul(ps_a2[:], lhsT=probs_sb[:], rhs=ones_col[:],
                                     start=True, stop=True)
                    nc.vector.tensor_tensor(aux_acc[:, 1:2], aux_acc[:, 1:2],
                                            ps_a2[:], ALU.add)

                    # ---- c^T row form for the per-expert broadcast ----
                    ps_ct = pms.tile([E, 128], f32, tag="ps_small")
                    nc.tensor.transpose(ps_ct[:], c_sb[:], ident[:])
                    nc.vector.tensor_copy(cT[:, bsl], ps_ct[:])

                # =====================  experts  =====================
                y_acc = [pyac.tile([128, T], f32, tag=f"yac{dc}", name=f"yac{dc}") for dc in range(ND)]
                for e in range(E):
                    # xe^T = xn^T * ln_w[e] + ln_b[e] (per-partition scale/bias)
                    xeT = [pxet.tile([128, T], f32r, tag=f"xet{dc}", name=f"xet{dc}") for dc in range(ND)]
                    for dc in range(ND):
                        col = np.s_[dc * E + e:dc * E + e + 1]
                        nc.scalar.activation(xeT[dc][:], xnT[dc][:], AF.Identity,
                                             scale=lnw_sb[:, col], bias=lnb_sb[:, col])

                    # ---- matmul1 + gelu: h^T chunks ----
                    hT = [pht.tile([128, T], f32r, tag=f"ht{hc}", name=f"ht{hc}") for hc in range(NH)]
                    for w1t in range(NW1):
                        w1sb = []
                        for dc in range(ND):
                            w = pw1.tile([128, W1TH], f32r, tag="w1", name="w1t")
                            nc.sync.dma_start(
                                w[:], w1_d[e, dc * 128:(dc + 1) * 128,
                                           w1t * W1TH:(w1t + 1) * W1TH].bitcast(f32r))
                            w1sb.append(w)
                        for hh in range(HPW):
                            hc = w1t * HPW + hh
                            ps_h = ph.tile([128, T], f32, tag="ps_h")
                            for dc in range(ND):
                                nc.tensor.matmul(
                                    ps_h[:],
                                    lhsT=w1sb[dc][:, hh * 128:(hh + 1) * 128],
                                    rhs=xeT[dc][:],
                                    start=(dc == 0), stop=(dc == ND - 1))
                            bcol = np.s_[hc * E + e:hc * E + e + 1]
                            nc.scalar.activation(hT[hc][:], ps_h[:], act_fn,
                                                 bias=b1t_sb[:, bcol])

                    # ---- combine-coefficient broadcast [128, T] for this expert ----
                    ps_cb = pms.tile([128, T], f32, tag="ps_small", name="ps_cb")
                    nc.tensor.matmul(ps_cb[:], lhsT=sel_sb[:, e * 128:(e + 1) * 128],
                                     rhs=cT[:], start=True, stop=True)
                    cb_sb = pcb.tile([128, T], f32, tag="cb")
                    nc.vector.tensor_copy(cb_sb[:], ps_cb[:])

                    # ---- matmul2 (+ scaled accumulation into y_acc) ----
                    for dg in range(NDG):
                        ps_y = [py.tile([128, T], f32, tag=f"ps_y{j}", name=f"ps_y{j}") for j in range(DG)]
                        for hc in range(NH):
                            w2sb = pw2.tile([128, DG * 128], f32r, tag="w2")
                            nc.sync.dma_start(
                                w2sb[:], w2_d[e, hc * 128:(hc + 1) * 128,
                                              dg * DG * 128:(dg + 1) * DG * 128].bitcast(f32r))
                            for j in range(DG):
                                nc.tensor.matmul(
                                    ps_y[j][:],
                                    lhsT=w2sb[:, j * 128:(j + 1) * 128],
                                    rhs=hT[hc][:],
                                    start=(hc == 0), stop=(hc == NH - 1))
                        for j in range(DG):
                            dc = dg * DG + j
                            if e == 0:
                                nc.vector.tensor_tensor(y_acc[dc][:], ps_y[j][:],
                                                        cb_sb[:], ALU.mult)
                            else:
                                tmp = ptmp.tile([128, T], f32, tag="tmp")
                                nc.vector.tensor_tensor(tmp[:], ps_y[j][:],
                                                        cb_sb[:], ALU.mult)
                                nc.vector.tensor_tensor(y_acc[dc][:], y_acc[dc][:],
                                                        tmp[:], ALU.add)

                # ---- + sum_e c_e * b2[e] : one K=E matmul per d-chunk ----
                for dc in range(ND):
                    ps_b2 = pms.tile([128, T], f32, tag="ps_small")
                    nc.tensor.matmul(ps_b2[:],
                                     lhsT=b2_sb[:, dc * 128:(dc + 1) * 128],
                                     rhs=cT[:], start=True, stop=True)
                    nc.vector.tensor_tensor(y_acc[dc][:], y_acc[dc][:],
                                            ps_b2[:], ALU.add)

                # ---- transpose y^T back to natural layout, store ----
                for s in range(NS):
                    row0 = (t * NS + s) * 128
                    ynat = pyout.tile([128, D], f32, tag="ynat")
                    for dc in range(ND):
                        ps_tr = pms.tile([128, 128], f32, tag="ps_small")
                        nc.tensor.transpose(ps_tr[:],
                                            y_acc[dc][:, s * 128:(s + 1) * 128],
                                            ident[:])
                        nc.vector.tensor_copy(ynat[:, dc * 128:(dc + 1) * 128],
                                              ps_tr[:])
                    nc.sync.dma_start(y_d[row0:row0 + 128, :], ynat[:])

            nc.sync.dma_start(aux_d[:], aux_acc[:])

    nc.compile()
    return nc


_KERNEL_CACHE = {}


def _get_kernel(key, **kw):
    if key not in _KERNEL_CACHE:
        _KERNEL_CACHE[key] = build_moe_kernel(**kw)
    return _KERNEL_CACHE[key]


def _prep(a):
    return np.ascontiguousarray(np.asarray(a, dtype=np.float32))


def _sel_const(E):
    sel = np.zeros((E, E * 128), np.float32)
    for e in range(E):
        sel[e, e * 128:(e + 1) * 128] = 1.0
    return sel


def kernel(x, gate_w, gate_b, ln_w, ln_b, w1, b1, w2, b2, top_k):
    from concourse.bass_utils import run_bass_kernel_spmd

    assert int(top_k) == 2
    x = _prep(x)
    B, D = x.shape
    E, _, H = np.asarray(w1).shape
    NC = 8
    BL = B // NC

    nc = _get_kernel(("full", BL, D, H, E), B_loc=BL, D=D, H=H, E=E, T=512)

    base = {
        "gwt": _prep(np.asarray(gate_w).T),
        "gb": _prep(np.asarray(gate_b).reshape(1, E)),
        "lnwt": _prep(np.asarray(ln_w).T),
        "lnbt": _prep(np.asarray(ln_b).T),
        "w1": _prep(w1),
        "b1t": _prep(np.asarray(b1).T),
        "w2": _prep(w2),
        "b2": _prep(b2),
        "sel": _sel_const(E),
    }
    in_maps = [dict(base, x=np.ascontiguousarray(x[i * BL:(i + 1) * BL]))
               for i in range(NC)]
    res = run_bass_kernel_spmd(nc, in_maps, core_ids=list(range(NC)))
    global LAST_EXEC_NS, LAST_RESULTS
    LAST_EXEC_NS = res.exec_time_ns
    LAST_RESULTS = res
    outs = res.results

    y = np.concatenate([o["y"] for o in outs], axis=0)
    probs = np.concatenate([o["probs"] for o in outs], axis=0)
    top_idx = np.concatenate([o["top_idx"] for o in outs], axis=0).astype(np.int32)
    top_w = np.concatenate([o["top_w"] for o in outs], axis=0)
    aux_p = np.stack([o["aux_part"] for o in outs])  # (NC, E, 2)
    counts = aux_p[:, :, 0].sum(axis=0)
    psums = aux_p[:, :, 1].sum(axis=0)
    aux = np.float32(E * np.sum((counts / (B * 2.0)) * (psums / B)))
    return y, probs, top_idx, top_w, aux
